# revision 25
# baseline (speedup 1.0000x reference)
"""Multi-head causal attention (B=2,S=2048,E=1024,H=16,D=64) on 8 NeuronCores.

Sharding: core c handles batch b=c//4 and head-group hg=c%4 (4 heads = 256
channels each).  Each core computes Q^T/K^T/V projections for its channel
slice, causal softmax attention for its 4 heads, and a partial output
projection through its slice of Wo.  Host sums the 4 partials per batch and
adds the bias.

v3 changes vs baseline:
  - Q^T/K^T and V projections in fp8(e4m3) DoubleRow (256-deep contraction
    per matmul -> half the matmul count); descale folded into the
    PSUM->SBUF cast.  V keys 0..127 stay bf16 (early queries average few
    keys, fp8 error there fails the gate).
  - Paired q/k tiles [128,S] (head pair on rows 0:64 / 64:128); scores use
    64-partition contraction => no zero-memsets, half the cast ops.
  - Score pairs share one [128,1024] PSUM tile; full off-diagonal pairs get
    a single merged exp (fewer ACT ops + semaphores -- ACT is the pacing
    engine).
  - Deeper AV lag (pend>=6) decouples PE from ACT latency; warm filler
    matmuls injected when real filler work runs out (HAM stays at 2.4GHz).
  - Output partials in fp16 (halves output DMA).

Attention dataflow (transpose-free):
  scores^T[k,q] = (K^T chunk)^T-stationary @ Q^T      (d on partitions, 64)
  P^T = exp(scores^T * D^-0.5)                        (ACT, causal-restricted)
  out^T[d,q] (+ denom row) = [V|1]^T-stationary @ P^T (k on partitions)
  normalize by broadcasted 1/denom, then
  partial[s,e] = (out_norm^T chunk)-stationary @ Wo-slice
"""

import sys

sys.path.insert(0, "/opt/trn_rl_repo")

import numpy as np

B, S, E, H, D = 2, 2048, 1024, 16, 64
N_CORES = 8
HPC = 4               # heads per core
CH = HPC * D          # 256 channels per core
SBK = 512             # seq block (moving free dim)
NSB = S // SBK        # 4
NE = E // 128         # 8 contraction chunks (bf16)
NC8 = 4               # fp8 DoubleRow contraction chunks (256 each)
NKC = S // 128        # 16 key chunks
XSC = 16.0            # host pre-scale on x (fp8)
WSC = 64.0            # host pre-scale on Wq/Wk/Wv (fp8)

_BUILT = {}


def _build():
    if "nc" in _BUILT:
        return _BUILT["nc"]

    from contextlib import ExitStack

    import concourse.bacc as bacc
    import concourse.tile as tile
    from concourse import mybir

    F32 = mybir.dt.float32
    F16 = mybir.dt.float16
    BF16 = mybir.dt.bfloat16
    F8 = mybir.dt.float8e4
    AF = mybir.ActivationFunctionType
    DR = mybir.MatmulPerfMode.DoubleRow

    nc = bacc.Bacc("TRN2", target_bir_lowering=False, debug=False,
                   num_devices=N_CORES)
    xt0 = nc.dram_tensor("xt0", [E, 128], BF16, kind="ExternalInput").ap()
    x8 = nc.dram_tensor("x8", [4 * 128, 2 * S], F8, kind="ExternalInput").ap()
    w8q = nc.dram_tensor("w8q", [4 * 128, 2 * CH], F8,
                         kind="ExternalInput").ap()
    w8k = nc.dram_tensor("w8k", [4 * 128, 2 * CH], F8,
                         kind="ExternalInput").ap()
    w8v = nc.dram_tensor("w8v", [4 * 128, 2 * CH], F8,
                         kind="ExternalInput").ap()
    wv = nc.dram_tensor("wv", [E, CH], BF16, kind="ExternalInput").ap()
    wo = nc.dram_tensor("wo", [CH, E], BF16, kind="ExternalInput").ap()
    tri = nc.dram_tensor("tri", [128, 128], BF16, kind="ExternalInput").ap()
    pout = nc.dram_tensor("pout", [S, E], F16, kind="ExternalOutput").ap()

    with tile.TileContext(nc) as tc, ExitStack() as ctx:
        wop = ctx.enter_context(tc.tile_pool(name="wop", bufs=2))
        qkp = ctx.enter_context(tc.tile_pool(name="qkp", bufs=4))
        vp = ctx.enter_context(tc.tile_pool(name="vp", bufs=NKC))
        trip = ctx.enter_context(tc.tile_pool(name="trip", bufs=1))
        pp = ctx.enter_context(tc.tile_pool(name="pp", bufs=2, space="PSUM"))
        sp = ctx.enter_context(tc.tile_pool(name="sp", bufs=2, space="PSUM"))
        avp = ctx.enter_context(tc.tile_pool(name="avp", bufs=2, space="PSUM"))
        xtp = ctx.enter_context(tc.tile_pool(name="xtp", bufs=NE))
        x8p = ctx.enter_context(tc.tile_pool(name="x8p", bufs=NC8))
        wp = ctx.enter_context(tc.tile_pool(name="wp", bufs=NE))
        w8p = ctx.enter_context(tc.tile_pool(name="w8p", bufs=3 * NC8))

        # --- loads: tri (warmup dep) + x8 first (critical path) ---
        tri_sb = trip.tile([128, 128], BF16, tag="tri")
        nc.sync.dma_start(tri_sb[:], tri[:, :])
        ones_sb = trip.tile([128, HPC], BF16, tag="ones")
        nc.vector.memset(ones_sb[:], 1.0)
        wrm = trip.tile([128, 128], BF16, tag="wrm")
        nc.vector.memset(wrm[:], 0.125)
        x8ts = []
        for c in range(NC8):
            t = x8p.tile([128, 2 * S], F8, tag="x8")
            nc.sync.dma_start(t[:], x8[c * 128:(c + 1) * 128, :])
            x8ts.append(t)
        xt0s = []
        for e in range(NE):
            t = xtp.tile([128, 128], BF16, tag="xt0")
            nc.sync.dma_start(t[:], xt0[e * 128:(e + 1) * 128, :])
            xt0s.append(t)
        w8qs, w8ks, w8vs = [], [], []
        for lst, srcw, tg in ((w8qs, w8q, "w8q"), (w8ks, w8k, "w8k"),
                              (w8vs, w8v, "w8v")):
            for c in range(NC8):
                t = w8p.tile([128, 2 * CH], F8, tag=tg)
                nc.gpsimd.dma_start(t[:], srcw[c * 128:(c + 1) * 128, :])
                lst.append(t)
        wvs = []
        for e in range(NE):
            t = wp.tile([128, CH], BF16, tag="wv")
            nc.gpsimd.dma_start(t[:], wv[e * 128:(e + 1) * 128, :])
            wvs.append(t)
        wos = []
        for cc in range(2):
            t = wop.tile([128, E], BF16, tag="wo")
            nc.gpsimd.dma_start(t[:], wo[cc * 128:(cc + 1) * 128, :])
            wos.append(t)

        onp = ctx.enter_context(tc.tile_pool(name="onp", bufs=2))
        ptp = ctx.enter_context(tc.tile_pool(name="ptp", bufs=5))
        recp = ctx.enter_context(tc.tile_pool(name="recp", bufs=2))
        bcp = ctx.enter_context(tc.tile_pool(name="bcp", bufs=2))
        oop = ctx.enter_context(tc.tile_pool(name="oop", bufs=3))
        ons = [onp.tile([128, S], BF16, tag="on", name=f"on{i}")
               for i in range(2)]

        # paired q/k tiles: head 2cc on rows 0:64, head 2cc+1 on rows 64:128
        qkt = {"q": [qkp.tile([128, S], BF16, tag="qk", name=f"qt{i}")
                     for i in range(2)],
               "k": [qkp.tile([128, S], BF16, tag="qk", name=f"kt{i}")
                     for i in range(2)]}
        vts = [vp.tile([128, HPC * 65], BF16, tag="v", name=f"v{i}")
               for i in range(NKC)]
        for t in vts:
            nc.vector.tensor_copy(
                t[:].rearrange("p (h c) -> p h c", h=HPC)[:, :, 64:65],
                ones_sb[:].unsqueeze(2))

        def warm_mm(n, pool=None):
            # reuse the pool's canonical tag so no extra PSUM slot is sized
            pool, tag = (sp, "sp") if pool is None else (pool, "pp")
            for _ in range(n):
                wps = pool.tile([128, 128], F32, tag=tag, name="warmps")
                nc.tensor.matmul(wps[:], lhsT=wrm[:], rhs=wrm[:],
                                 start=True, stop=True)

        # ---- dense-matmul group emitters (filler work) ----
        def qk_group(name, wts, cc, sb, sprinkle=0):
            ps = pp.tile([128, SBK], F32, tag="pp", name=f"ps_{name}{cc}{sb}")
            for c in range(NC8):
                lhsT = wts[c][:].rearrange("p (two ch) -> p two ch",
                                           two=2)[:, :, cc * 128:(cc + 1) * 128]
                rhs = x8ts[c][:].rearrange("p (two s) -> p two s",
                                           two=2)[:, :, sb * SBK:(sb + 1) * SBK]
                nc.tensor.matmul(ps[:], lhsT=lhsT, rhs=rhs,
                                 start=(c == 0), stop=(c == NC8 - 1),
                                 perf_mode=DR)
                if sprinkle:
                    warm_mm(sprinkle)
            cols = slice(sb * SBK, (sb + 1) * SBK)
            nc.vector.tensor_scalar_mul(qkt[name][cc][:, cols], ps[:],
                                        1.0 / (XSC * WSC))

        def v_group(sc, sprinkle=0):
            ps = pp.tile([128, CH], F32, tag="pp", name=f"ps_v{sc}")
            if sc == 0:
                # bf16: early queries average few keys; fp8 V fails there
                for e in range(NE):
                    nc.tensor.matmul(ps[:], lhsT=xt0s[e][:],
                                     rhs=wvs[e][:], start=(e == 0),
                                     stop=(e == NE - 1))
                    if sprinkle:
                        warm_mm(sprinkle)
            else:
                for c in range(NC8):
                    lhsT = x8ts[c][:].rearrange(
                        "p (two s) -> p two s",
                        two=2)[:, :, sc * 128:(sc + 1) * 128]
                    rhs = w8vs[c][:].rearrange("p (two ch) -> p two ch", two=2)
                    nc.tensor.matmul(ps[:], lhsT=lhsT, rhs=rhs,
                                     start=(c == 0), stop=(c == NC8 - 1),
                                     perf_mode=DR)
                    if sprinkle:
                        warm_mm(sprinkle)
            psv = ps[:].rearrange("p (h c) -> p h c", h=HPC)
            dst = vts[sc][:].rearrange("p (h c) -> p h c", h=HPC)
            if sc == 0:
                nc.vector.tensor_copy(dst[:, :, 0:64], psv)
            else:
                nc.vector.tensor_scalar_mul(dst[:, :, 0:64], psv,
                                            1.0 / (XSC * WSC))

        woh = {}

        def wo_half(sc, eb):
            ps = pp.tile([128, SBK], F32, tag="pp", name=f"ph_o{sc}{eb}")
            nc.tensor.matmul(ps[:], lhsT=ons[0][:, sc * 128:(sc + 1) * 128],
                             rhs=wos[0][:, eb * SBK:(eb + 1) * SBK],
                             start=True, stop=True)
            t = oop.tile([128, SBK], F32, tag="oh", bufs=8,
                         name=f"oh{sc}{eb}")
            nc.vector.tensor_copy(t[:], ps[:])
            woh[(sc, eb)] = t

        def wo_combine(sc, eb):
            ps = pp.tile([128, SBK], F32, tag="pp", name=f"pc_o{sc}{eb}")
            nc.tensor.matmul(ps[:], lhsT=ons[1][:, sc * 128:(sc + 1) * 128],
                             rhs=wos[1][:, eb * SBK:(eb + 1) * SBK],
                             start=True, stop=True)
            oo = oop.tile([128, SBK], F16, tag="oo", name=f"oc{sc}{eb}")
            nc.vector.tensor_add(oo[:], ps[:], woh[(sc, eb)][:])
            nc.sync.dma_start(
                pout[sc * 128:(sc + 1) * 128, eb * SBK:(eb + 1) * SBK],
                oo[:])

        def wo_group(sc, eb):
            ps = pp.tile([128, SBK], F32, tag="pp", name=f"ps_o{sc}{eb}")
            for cc in range(2):
                nc.tensor.matmul(ps[:],
                                 lhsT=ons[cc][:, sc * 128:(sc + 1) * 128],
                                 rhs=wos[cc][:, eb * SBK:(eb + 1) * SBK],
                                 start=(cc == 0), stop=(cc == 1))
            oo = oop.tile([128, SBK], F16, tag="oo", name=f"oo{sc}{eb}")
            nc.vector.tensor_copy(oo[:], ps[:])
            nc.sync.dma_start(
                pout[sc * 128:(sc + 1) * 128, eb * SBK:(eb + 1) * SBK],
                oo[:])

        from collections import deque
        fillers = deque()
        warm_mm(40)

        # prologue: everything attention (qb0, h0/h1) needs
        for sb in range(NSB):
            qk_group("q", w8qs, 0, sb, sprinkle=2)
        for sb in range(NSB):
            qk_group("k", w8ks, 0, sb, sprinkle=2)
        for sc in range(4):
            v_group(sc)
        # filler consumed during qb0: projections for heads 2-3, V for qb1
        for sb in range(NSB):
            fillers.append(lambda sb=sb: qk_group("q", w8qs, 1, sb))
        for sb in range(NSB):
            fillers.append(lambda sb=sb: qk_group("k", w8ks, 1, sb))
        for sc in range(4, 8):
            fillers.append(lambda sc=sc: v_group(sc))

        # ---- attention: qb-outer so Wo/output-DMA spread across the run ----
        for qb in range(NSB):
            if qb >= 1:
                for sc in range(4 * (qb + 1), 4 * (qb + 2)):
                    if sc < NKC:
                        fillers.append(lambda sc=sc: v_group(sc))
                for sc in range(4 * (qb - 1), 4 * qb):
                    for eb in range(2):
                        fillers.append(
                            lambda sc=sc, eb=eb: wo_group(sc, eb))
            for h in range(HPC):
                last_head = (qb == NSB - 1 and h == HPC - 1)
                if qb == NSB - 1 and h == 2:
                    # h0/h1 of the last q-block are done: their half of the
                    # final output projection can overlap h2/h3
                    for sc in range(12, NKC):
                        for eb in range(2):
                            fillers.append(
                                lambda sc=sc, eb=eb: wo_half(sc, eb))
                cc, po = h // 2, (h % 2) * 64
                qt, kt = qkt["q"][cc], qkt["k"][cc]
                av = avp.tile([65, SBK], F32, tag="av", name=f"av{qb}{h}")
                nk = 4 * (qb + 1)
                pend = deque()  # AV lags scores; emit in same-shape pairs
                def flush_av(nmax, final=False):
                    n = 0
                    while pend and n < nmax:
                        pkc, pbase, pj0, ppt = pend.popleft()
                        nc.tensor.matmul(
                            av[:, pj0:SBK],
                            lhsT=vts[pkc][:, h * 65:(h + 1) * 65],
                            rhs=ppt[:, pbase + pj0:pbase + SBK],
                            start=(pkc == 0),
                            stop=(final and not pend),
                            skip_group_check=True)
                        n += 1
                for kc2 in range(0, nk, 2):
                    j0s = [max(0, (kc2 + i) * 128 - qb * SBK)
                           for i in range(2)]
                    ss = sp.tile([128, 2 * SBK], F32, tag="sp")
                    pt = ptp.tile([128, 2 * SBK], BF16, tag="pt", name="pt")
                    for i, kc in enumerate((kc2, kc2 + 1)):
                        k0, j0, base = kc * 128, j0s[i], i * SBK
                        nc.tensor.matmul(
                            ss[:, base + j0:base + SBK],
                            lhsT=kt[po:po + 64, k0:k0 + 128],
                            rhs=qt[po:po + 64, qb * SBK + j0:(qb + 1) * SBK],
                            start=True, stop=True,
                            skip_group_check=(i == 1))
                        if fillers:
                            fillers.popleft()()
                        elif not last_head:
                            warm_mm(1, pool=pp)
                    if j0s[1] == 0:  # both full: one merged exp
                        nc.scalar.activation(pt[:, :], ss[:, :], AF.Exp,
                                             scale=float(D) ** -0.5)
                    else:
                        for i in range(2):
                            j0, base = j0s[i], i * SBK
                            nc.scalar.activation(
                                pt[:, base + j0:base + SBK],
                                ss[:, base + j0:base + SBK],
                                AF.Exp, scale=float(D) ** -0.5)
                    for i, kc in enumerate((kc2, kc2 + 1)):
                        k0, j0, base = kc * 128, j0s[i], i * SBK
                        if k0 >= qb * SBK:  # diag chunk: mask 128-wide band
                            nc.vector.tensor_mul(
                                pt[:, base + j0:base + j0 + 128],
                                pt[:, base + j0:base + j0 + 128], tri_sb[:])
                        pend.append((kc, base, j0, pt))
                    if len(pend) >= 6:
                        flush_av(2)
                if last_head:
                    warm_mm(4)
                flush_av(99, final=True)
                if last_head:
                    # cover the final normalize latency so the tail
                    # combines start at full clock
                    warm_mm(10)
                rec = recp.tile([1, SBK], F32, tag="rec")
                nc.vector.tensor_copy(rec[:], av[64:65, :])
                rec2 = recp.tile([1, SBK], F32, tag="rec2")
                nc.vector.reciprocal_approx_fast(rec2[:], rec[:])
                bc = bcp.tile([64, SBK], F32, tag="bc")
                nc.gpsimd.partition_broadcast(bc[:], rec2[:])
                nc.vector.tensor_mul(
                    ons[cc][po:po + 64, qb * SBK:(qb + 1) * SBK],
                    av[0:64, :], bc[:])

        while fillers:
            fillers.popleft()()
        # tail: combine the remaining half of the last q-block's projection
        for sc in range(12, NKC):
            for eb in range(2):
                wo_combine(sc, eb)


    nc.compile()
    _BUILT["nc"] = nc
    return nc


def _install_ntff_shim():
    """Provide antenv.axon_hooks (missing in this image) so trace=True works."""
    import types
    try:
        from antenv.axon_hooks import get_axon_ntff_profile_hook  # noqa: F401
        return
    except ImportError:
        pass
    import antenv
    from trn_agent_boot.trn_boot import _ntff_profile_via_ctypes
    hook = _ntff_profile_via_ctypes("/opt/axon/libaxon_pjrt.so")
    mod = types.ModuleType("antenv.axon_hooks")
    mod._hook = hook
    mod.get_axon_ntff_profile_hook = lambda: mod._hook
    mod.set_axon_ntff_profile_hook = lambda h: setattr(mod, "_hook", h)
    sys.modules["antenv.axon_hooks"] = mod
    antenv.axon_hooks = mod


def _pack_pairs(arr):
    """[E, N] -> [512, 2N]: chunk c row p holds pair (c*256+p, c*256+128+p)."""
    Edim, N = arr.shape
    return np.ascontiguousarray(
        arr.reshape(4, 2, 128, N).transpose(0, 2, 1, 3).reshape(512, 2 * N))


def kernel(x, Wq, Wk, Wv, Wo, bo, _trace=False):
    from concourse.bass_utils import run_bass_kernel_spmd

    nc = _build()

    x = np.asarray(x, dtype=np.float32)
    Wq = np.asarray(Wq, dtype=np.float32)
    Wk = np.asarray(Wk, dtype=np.float32)
    Wv = np.asarray(Wv, dtype=np.float32)
    Wo = np.asarray(Wo, dtype=np.float32)
    bo = np.asarray(bo, dtype=np.float32)

    import ml_dtypes
    bf = ml_dtypes.bfloat16
    f8 = ml_dtypes.float8_e4m3fn
    tri = np.triu(np.ones((128, 128), dtype=np.float32)).astype(bf)
    xt_b = [np.ascontiguousarray(x[b].T) for b in range(B)]
    x8_b = [_pack_pairs(xt_b[b] * XSC).astype(f8) for b in range(B)]
    in_maps = []
    for c in range(N_CORES):
        b, hg = c // HPC, c % HPC
        sl = slice(hg * CH, (hg + 1) * CH)
        in_maps.append({
            "xt0": np.ascontiguousarray(xt_b[b][:, 0:128]).astype(bf),
            "x8": x8_b[b],
            "w8q": _pack_pairs(Wq[:, sl] * WSC).astype(f8),
            "w8k": _pack_pairs(Wk[:, sl] * WSC).astype(f8),
            "w8v": _pack_pairs(Wv[:, sl] * WSC).astype(f8),
            "wv": np.ascontiguousarray(Wv[:, sl]).astype(bf),
            "wo": np.ascontiguousarray(Wo[sl, :]).astype(bf),
            "tri": tri,
        })

    kwargs = {}
    if _trace:
        _install_ntff_shim()
        kwargs = dict(trace=True, trace_cores=[0])
    res = run_bass_kernel_spmd(nc, in_maps, core_ids=list(range(N_CORES)),
                               **kwargs)

    out = np.zeros((B, S, E), dtype=np.float32)
    for c in range(N_CORES):
        out[c // HPC] += res.results[c]["pout"].astype(np.float32)
    out += bo
    if _trace:
        return out, res
    return out
